# revision 31
# baseline (speedup 1.0000x reference)
"""Decoder block (LN1 -> causal MHA -> LN2 -> GELU FFN, residuals) on 8 NeuronCores.

Sharding: 2-way data parallel over batch x 4-way tensor parallel over heads.
Core c: batch b=c//4, heads [4*(c%4) .. 4*(c%4)+4); after an AllToAll of
per-head attention context, core c owns token slice [512*(c%4) .. +512) of its
batch for out-proj / LN2 / FFN.

Activations live in transposed layout [feature, token]; all large matmuls run
in bfloat16 (fast weight load, fp32 PSUM accumulate).  The AllToAll is split
in two (heads 0-1 / heads 2-3): the first overlaps the second half of
attention, the second overlaps the first out-projection pass.

SBUF slab tags shared across phases:
  slabW [128, 4096] bf16: wk+wv (phase 1)  ->  x2r (phases 4-6)
  slabE [128, 4160] bf16: V'   (phases 1-3) ->  cf (phase 4+)
"""
import sys
import numpy as np

sys.path.insert(0, '/opt/trn_rl_repo')

import ml_dtypes                       # noqa: E402
import concourse.bass as bass          # noqa: E402
import concourse.bacc as bacc          # noqa: E402
import concourse.tile as tile          # noqa: E402
from concourse import mybir            # noqa: E402
from concourse.masks import make_identity  # noqa: E402
from concourse.bass_utils import run_bass_kernel_spmd  # noqa: E402

F32 = mybir.dt.float32
F32R = mybir.dt.float32r
HF = mybir.dt.bfloat16
BF_NP = ml_dtypes.bfloat16
AF = mybir.ActivationFunctionType
ALU = mybir.AluOpType

B, S, E, H, D, F = 2, 2048, 1024, 16, 64, 4096
NC = 8
T = S
TS = 512
EPS = 1e-5
NEH = E // 128         # 8
NFH = F // 128         # 32
HPC = 4                # heads per core
MCH = 2                # d-chunks for 4 heads
LW = 512               # layernorm / projection chunk width
NCH = T // LW          # 4 token chunks
FG = 8                 # fc1/fc2 f-chunks per interleaved group


def build(causal=True):
    nc = bacc.Bacc("TRN2", target_bir_lowering=False, debug=False, num_devices=NC)

    xT_d = nc.dram_tensor("xT", [E, T], HF, kind="ExternalInput").ap()
    wq_d = nc.dram_tensor("wq", [E, HPC * D], HF, kind="ExternalInput").ap()
    wk_d = nc.dram_tensor("wk", [E, HPC * D], HF, kind="ExternalInput").ap()
    wv_d = nc.dram_tensor("wv", [E, HPC * D], HF, kind="ExternalInput").ap()
    bq_d = nc.dram_tensor("bq", [128, MCH], F32, kind="ExternalInput").ap()
    bk_d = nc.dram_tensor("bk", [128, MCH], F32, kind="ExternalInput").ap()
    bv_d = nc.dram_tensor("bv", [1, HPC * D], HF, kind="ExternalInput").ap()
    wot_d = nc.dram_tensor("wot", [NEH, 128, E], HF, kind="ExternalInput").ap()
    bo_d = nc.dram_tensor("bo", [128, NEH], F32, kind="ExternalInput").ap()
    w1_d = nc.dram_tensor("w1", [NFH, 128, E], HF, kind="ExternalInput").ap()
    b1_d = nc.dram_tensor("b1", [128, NFH], F32, kind="ExternalInput").ap()
    w2_d = nc.dram_tensor("w2", [F, E], HF, kind="ExternalInput").ap()
    b2_d = nc.dram_tensor("b2", [128, NEH], F32, kind="ExternalInput").ap()
    g2c_d = nc.dram_tensor("g2c", [128, NEH], F32, kind="ExternalInput").ap()
    b2c_d = nc.dram_tensor("b2c", [128, NEH], F32, kind="ExternalInput").ap()
    xres_d = nc.dram_tensor("xres", [E, TS], F32, kind="ExternalInput").ap()
    zm_d = nc.dram_tensor("zm", [128, NC], F32, kind="ExternalInput").ap()
    cmask_d = nc.dram_tensor("cmask", [128, 4 * 512], HF, kind="ExternalInput").ap()
    onch_d = nc.dram_tensor("onch", [128, 1], HF, kind="ExternalInput").ap()
    onrr_d = nc.dram_tensor("onrr", [1, 128], F32R, kind="ExternalInput").ap()
    onrh_d = nc.dram_tensor("onrh", [1, 128], HF, kind="ExternalInput").ap()
    on4_d = nc.dram_tensor("on4", [128, HPC], HF, kind="ExternalInput").ap()
    if not causal:
        mT_d = nc.dram_tensor("mT", [S, S], F32, kind="ExternalInput").ap()
    out_d = nc.dram_tensor("out", [TS, E], F32, kind="ExternalOutput").ap()

    with tile.TileContext(nc) as tc:
        with (
            tc.tile_pool(name="res", bufs=1) as res,
            tc.tile_pool(name="wk_", bufs=2) as wrk,
            tc.tile_pool(name="st", bufs=1) as st,
            tc.tile_pool(name="ps", bufs=2, space="PSUM") as ps,
            tc.tile_pool(name="dram", bufs=1, space="DRAM") as dram,
        ):
            def rtile(name, shape, dt, tag=None):
                return res.tile(shape, dt, tag=tag or name, name=name)

            def pA(name, shape=(128, 1024)):
                return ps.tile(list(shape), F32, tag="pA", name=name, bufs=3)

            def pB(name, shape=(128, 512)):
                return ps.tile(list(shape), F32, tag="pB", name=name, bufs=2)

            # ---- constants ----
            onch = rtile("onch", [128, 1], HF)
            nc.sync.dma_start(onch[:], onch_d[:])
            onrr = rtile("onrr", [1, 128], F32R)
            nc.sync.dma_start(onrr[:], onrr_d[:])
            onrh = rtile("onrh", [1, 128], HF)
            nc.sync.dma_start(onrh[:], onrh_d[:])
            on4 = rtile("on4", [128, HPC], HF)
            nc.sync.dma_start(on4[:], on4_d[:])
            ident = rtile("ident", [128, 128], F32)
            make_identity(nc, ident[:])
            zm = rtile("zm", [128, NC], F32)
            nc.sync.dma_start(zm[:], zm_d[:])
            cmask = rtile("cmask", [128, 4 * 512], HF)
            nc.sync.dma_start(cmask[:], cmask_d[:])
            bq = rtile("bq", [128, MCH], F32)
            nc.sync.dma_start(bq[:], bq_d[:])
            bk = rtile("bk", [128, MCH], F32)
            nc.sync.dma_start(bk[:], bk_d[:])
            bv = rtile("bv", [1, HPC * D], HF)
            nc.sync.dma_start(bv[:], bv_d[:])
            bo = rtile("bo", [128, NEH], F32)
            nc.sync.dma_start(bo[:], bo_d[:])
            b1 = rtile("b1", [128, NFH], F32)
            nc.sync.dma_start(b1[:], b1_d[:])
            b2 = rtile("b2", [128, NEH], F32)
            nc.sync.dma_start(b2[:], b2_d[:])
            g2c = rtile("g2c", [128, NEH], F32)
            nc.sync.dma_start(g2c[:], g2c_d[:])
            b2c = rtile("b2c", [128, NEH], F32)
            nc.sync.dma_start(b2c[:], b2c_d[:])

            # ---- resident weights/tensors ----
            wqb = rtile("wqb", [128, NEH * HPC * D], HF)        # [128, 2048]
            for e in range(NEH):
                nc.sync.dma_start(wqb[:, 256 * e:256 * e + 256],
                                  wq_d[128 * e:128 * e + 128, :])
            wkvb = rtile("wkvb", [128, 4096], HF, tag="slabW")
            for e in range(NEH):
                nc.sync.dma_start(wkvb[:, 256 * e:256 * e + 256],
                                  wk_d[128 * e:128 * e + 128, :])
                nc.sync.dma_start(wkvb[:, 2048 + 256 * e:2048 + 256 * e + 256],
                                  wv_d[128 * e:128 * e + 128, :])
            qtb = rtile("qtb", [128, MCH * T], HF)              # [128, 4096]
            ktb = rtile("ktb", [128, MCH * T], HF)              # [128, 4096]
            vpb = rtile("vpb", [128, (T // 128) * HPC * 65], HF, tag="slabE")
            cfb = rtile("cfb", [128, 4096], HF)

            def vsl(tt):
                return vpb[:, 260 * tt:260 * tt + 260]

            # ---- layernorm helpers ----
            def ln_chain(pxq, width, idx):
                """DVE stats chain on accumulated [1,1024] (sum|sumsq) PSUM tile;
                returns (a_row, c_row) [1,width] f32r tiles (rstd, -mu*rstd)."""
                mean = st.tile([1, width], F32, tag="mean", name=f"mean{idx}")
                nc.vector.tensor_scalar_mul(mean[:], pxq[:, 0:width], 1.0 / E)
                msq = st.tile([1, width], F32, tag="msq", name=f"msq{idx}")
                nc.vector.tensor_scalar_mul(msq[:], pxq[:, 512:512 + width], 1.0 / E)
                scr = st.tile([1, width], F32, tag="scr", name=f"scr{idx}")
                nc.vector.tensor_tensor(scr[:], mean[:], mean[:], ALU.mult)
                var = st.tile([1, width], F32, tag="var", name=f"var{idx}")
                nc.vector.scalar_tensor_tensor(
                    out=var[:], in0=msq[:], scalar=float(EPS), in1=scr[:],
                    op0=ALU.add, op1=ALU.subtract)
                std = st.tile([1, width], F32, tag="std", name=f"std{idx}")
                nc.scalar.activation(std[:], var[:], AF.Sqrt)
                rstd = st.tile([1, width], F32, tag="rstd", name=f"rstd{idx}")
                nc.vector.reciprocal_approx_fast(rstd[:], std[:])
                a_row = st.tile([1, width], F32R, tag="a_row", name=f"a_row{idx}")
                nc.vector.tensor_copy(a_row[:], rstd[:])
                ct = st.tile([1, width], F32, tag="ct", name=f"ct{idx}")
                nc.vector.tensor_tensor(ct[:], mean[:], rstd[:], ALU.mult)
                c_row = st.tile([1, width], F32R, tag="c_row", name=f"c_row{idx}")
                nc.vector.tensor_scalar_mul(c_row[:], ct[:], -1.0)
                return a_row, c_row

            def ln_bc(a_row, c_row, width, idx):
                """Broadcast a/c rows into a bf16 SBUF tile [128, 1024]:
                [:, :512]=a, [:, 512:]=c."""
                bc = pA(f"bc{idx}")
                nc.tensor.matmul(bc[:, 0:width], onrr[:], a_row[:],
                                 start=True, stop=True)
                nc.tensor.matmul(bc[:, 512:512 + width], onrr[:], c_row[:],
                                 start=True, stop=True)
                bch = wrk.tile([128, 1024], HF, tag="bch", name=f"bch{idx}", bufs=2)
                nc.scalar.copy(bch[:, 0:width], bc[:, 0:width])
                nc.scalar.copy(bch[:, 512:512 + width], bc[:, 512:512 + width])
                return bch

            # ---- phase 1: LN1 fused with Q/K/V projections, per 512 tokens ----
            # Chunk pipeline: chunk t's zz (DVE) production is interleaved
            # per-feature-chunk with its m=0 projections and chunk t+1's LN
            # sums on PE, so neither engine waits on the other's full chunk.
            xt_all, zz_all, pxq_all, row_all = [], [], [], []

            def p1_load(tcn):
                xtc = wrk.tile([128, NEH * LW], HF, tag="xt", name=f"xt{tcn}",
                               bufs=3)
                for u in range(2):
                    nc.sync.dma_start(
                        xtc[:, 2048 * u:2048 * u + 2048]
                        .rearrange("p (e w) -> p e w", e=4),
                        xT_d[512 * u:512 * u + 512, LW * tcn:LW * tcn + LW]
                        .rearrange("(e p) w -> p e w", p=128))
                xt_all.append(xtc)

            def p1_sums_e(tcn, e):
                if e == 0:
                    pxq_all.append(pA(f"pxq{tcn}", (1, 1024)))
                pxq = pxq_all[tcn]
                src = xt_all[tcn][:, 512 * e:512 * e + 512]
                sq = wrk.tile([128, LW], HF, tag="sqh", name=f"sq{tcn}_{e}",
                              bufs=3)
                nc.scalar.activation(sq[:], src, AF.Square)
                nc.tensor.matmul(pxq[:, 512:1024], onch[:], sq[:],
                                 start=(e == 0), stop=(e == NEH - 1))
                nc.tensor.matmul(pxq[:, 0:512], onch[:], src,
                                 start=(e == 0), stop=(e == NEH - 1))

            def p1_chain(tcn):
                rows = ln_chain(pxq_all[tcn], LW, f"l1c{tcn}")
                row_all.append(rows)

            def p1_zz_e(tcn, e, bch):
                tmp = wrk.tile([128, LW], HF, tag="lnt", name=f"lnt{tcn}_{e}",
                               bufs=3)
                nc.vector.tensor_tensor(tmp[:], xt_all[tcn][:, 512 * e:512 * e + 512],
                                        bch[:, 0:LW], ALU.mult)
                zz = wrk.tile([128, LW], HF, tag="zz", name=f"zz{tcn}_{e}",
                              bufs=10)
                nc.vector.tensor_tensor(zz[:], tmp[:], bch[:, 512:512 + LW],
                                        ALU.add)
                zz_all[tcn].append(zz)

            p1_load(0)
            p1_load(1)
            for e in range(NEH):
                p1_sums_e(0, e)
            p1_chain(0)
            for tcn in range(NCH):
                if tcn + 2 < NCH:
                    p1_load(tcn + 2)
                a_row, c_row = row_all[tcn]
                bch = ln_bc(a_row, c_row, LW, f"l1c{tcn}")
                zz_all.append([])
                zz_l = zz_all[tcn]
                pq0 = pB(f"pq{tcn}_0", (128, LW))
                pk0 = pB(f"pk{tcn}_0", (128, LW))
                for e in range(NEH):
                    p1_zz_e(tcn, e, bch)
                    nc.tensor.matmul(pq0[:], wqb[:, 256 * e:256 * e + 128],
                                     zz_l[e][:], start=(e == 0), stop=(e == NEH - 1))
                    nc.tensor.matmul(pk0[:], wkvb[:, 256 * e:256 * e + 128],
                                     zz_l[e][:], start=(e == 0), stop=(e == NEH - 1))
                    if tcn + 1 < NCH:
                        p1_sums_e(tcn + 1, e)
                c0 = LW * tcn
                nc.vector.tensor_scalar_add(qtb[:, c0:c0 + LW], pq0[:],
                                            bq[:, 0:1])
                nc.vector.tensor_scalar_add(ktb[:, c0:c0 + LW], pk0[:],
                                            bk[:, 0:1])
                pq1 = pB(f"pq{tcn}_1", (128, LW))
                for e in range(NEH):
                    nc.tensor.matmul(pq1[:], wqb[:, 256 * e + 128:256 * e + 256],
                                     zz_l[e][:], start=(e == 0), stop=(e == NEH - 1))
                pk1 = pB(f"pk{tcn}_1", (128, LW))
                for e in range(NEH):
                    nc.tensor.matmul(pk1[:], wkvb[:, 256 * e + 128:256 * e + 256],
                                     zz_l[e][:], start=(e == 0), stop=(e == NEH - 1))
                c1 = 2048 + LW * tcn
                nc.vector.tensor_scalar_add(qtb[:, c1:c1 + LW], pq1[:],
                                            bq[:, 1:2])
                nc.vector.tensor_scalar_add(ktb[:, c1:c1 + LW], pk1[:],
                                            bk[:, 1:2])
                for sub in range(LW // 128):
                    tt = (LW * tcn) // 128 + sub
                    pv = pB(f"pv{tt}", (128, HPC * D))
                    for e in range(NEH):
                        nc.tensor.matmul(
                            pv[:], zz_l[e][:, 128 * sub:128 * sub + 128],
                            wkvb[:, 2048 + 256 * e:2048 + 256 * e + 256],
                            start=(e == 0), stop=False)
                    nc.tensor.matmul(pv[:], onrh[:], bv[:], start=False, stop=True)
                    nc.vector.tensor_copy(
                        vsl(tt).rearrange("p (h d) -> p h d", h=HPC)[:, :, 0:D],
                        pv[:].rearrange("p (h d) -> p h d", h=HPC))
                    nc.vector.tensor_copy(
                        vsl(tt).rearrange("p (h d) -> p h d", h=HPC)[:, :, D:D + 1],
                        on4[:].rearrange("p (h o) -> p h o", o=1))
                if tcn + 1 < NCH:
                    p1_chain(tcn + 1)

            # ---- phase 3: attention (h-major; A2A #1 after heads 0-1) ----
            stageA = dram.tile([NC, 2 * D, 512], HF, tag="stageA", name="stageA")
            stageB = dram.tile([NC, 2 * D, 512], HF, tag="stageB", name="stageB")
            stage2A = dram.tile([NC, 2 * D, 512], HF, tag="stage2A", name="stage2A")
            stage2B = dram.tile([NC, 2 * D, 512], HF, tag="stage2B", name="stage2B")
            NSB = T // 512
            for h in range(HPC):
                for i in range(NSB):
                    m, hp = h // 2, h % 2
                    qsl = qtb[64 * hp:64 * hp + 64, 2048 * m + 512 * i:
                              2048 * m + 512 * i + 512]
                    n_kc = 4 * (i + 1) if causal else T // 128
                    psav = pB(f"av{h}_{i}")
                    ngr = (n_kc + 1) // 2
                    psc_l = {}

                    def emit_qk(g, h=h, i=i, m=m, hp=hp, qsl=qsl, n_kc=n_kc,
                                psc_l=psc_l):
                        kcs = [k for k in (2 * g, 2 * g + 1) if k < n_kc]
                        psc = pA(f"sc{h}_{i}_{g}")
                        for u, kc in enumerate(kcs):
                            nc.tensor.matmul(
                                psc[:, 512 * u:512 * u + 512],
                                ktb[64 * hp:64 * hp + 64,
                                    2048 * m + 128 * kc:2048 * m + 128 * kc + 128],
                                qsl, start=True, stop=True)
                        if not causal:
                            for u, kc in enumerate(kcs):
                                mb = wrk.tile([128, 512], F32, tag="mb",
                                              name=f"mb{h}_{i}_{g}_{u}", bufs=2)
                                nc.sync.dma_start(
                                    mb[:], mT_d[128 * kc:128 * kc + 128,
                                                512 * i:512 * i + 512])
                                nc.vector.tensor_tensor(
                                    psc[:, 512 * u:512 * u + 512],
                                    psc[:, 512 * u:512 * u + 512], mb[:], ALU.add)
                        psc_l[g] = (psc, kcs)

                    emit_qk(0)
                    for g in range(ngr):
                        if g + 1 < ngr:
                            emit_qk(g + 1)
                        psc, kcs = psc_l.pop(g)
                        eg = wrk.tile([128, 1024], HF, tag="exp",
                                      name=f"exp{h}_{i}_{g}", bufs=3)
                        w = 512 * len(kcs)
                        nc.scalar.activation(eg[:, 0:w], psc[:, 0:w], AF.Exp)
                        if causal:
                            for u, kc in enumerate(kcs):
                                j = kc - 4 * i
                                if j >= 0:
                                    nc.vector.tensor_tensor(
                                        eg[:, 512 * u:512 * u + 512],
                                        eg[:, 512 * u:512 * u + 512],
                                        cmask[:, 512 * j:512 * j + 512], ALU.mult)
                        for u, kc in enumerate(kcs):
                            nc.tensor.matmul(
                                psav[0:65, :], vsl(kc)[:, 65 * h:65 * h + 65],
                                eg[:, 512 * u:512 * u + 512],
                                start=(kc == 0), stop=(kc == n_kc - 1),
                                skip_group_check=True)
                    sden = st.tile([1, 512], F32, tag="sden", name=f"sden{h}_{i}")
                    nc.scalar.copy(sden[:], psav[64:65, :])
                    rc = st.tile([1, 512], F32, tag="rc", name=f"rc{h}_{i}")
                    nc.vector.reciprocal_approx_fast(rc[:], sden[:])
                    rb = wrk.tile([64, 512], F32, tag="rb", name=f"rb{h}_{i}", bufs=2)
                    nc.gpsimd.partition_broadcast(rb[:], rc[:])
                    ctx = wrk.tile([64, 512], HF, tag="ctx", name=f"ctx{h}_{i}",
                                   bufs=3)
                    nc.vector.tensor_tensor(ctx[:], psav[0:64, :], rb[:], ALU.mult)
                    stg = stageA if h < 2 else stageB
                    hh = h % 2
                    for s_ in (i, i + 4):
                        cz = wrk.tile([64, 512], HF, tag="cz",
                                      name=f"cz{h}_{i}_{s_}", bufs=3)
                        nc.vector.tensor_scalar_mul(cz[:], ctx[:],
                                                    zm[0:64, s_:s_ + 1])
                        nc.sync.dma_start(stg[s_][64 * hh:64 * hh + 64, :], cz[:])
                if h == 1:
                    nc.gpsimd.collective_compute(
                        "AllToAll", ALU.bypass, replica_groups=[list(range(NC))],
                        ins=[stageA.opt()], outs=[stage2A.opt()])
                if h == 2:
                    # pull the A2A#1 results in on the gpsimd queue while
                    # head 3's attention still runs on PE/ACT/DVE/Sync
                    for s in range(4):
                        sa = wrk.tile([128, 512], HF, tag="sa", name=f"saA{s}",
                                      bufs=2)
                        nc.gpsimd.dma_start(sa[:], stage2A[s])
                        sb_ = wrk.tile([128, 512], HF, tag="sa", name=f"sbA{s}",
                                       bufs=2)
                        nc.gpsimd.dma_start(sb_[:], stage2A[4 + s])
                        nc.gpsimd.tensor_tensor(cfb[:, 512 * s:512 * s + 512],
                                                sa[:], sb_[:], ALU.add)
            nc.gpsimd.collective_compute(
                "AllToAll", ALU.bypass, replica_groups=[list(range(NC))],
                ins=[stageB.opt()], outs=[stage2B.opt()])
            for s in range(4):
                sa = wrk.tile([128, 512], HF, tag="sa", name=f"saB{s}", bufs=2)
                nc.gpsimd.dma_start(sa[:], stage2B[s])
                sb_ = wrk.tile([128, 512], HF, tag="sa", name=f"sbB{s}", bufs=2)
                nc.gpsimd.dma_start(sb_[:], stage2B[4 + s])
                nc.gpsimd.tensor_tensor(cfb[:, 2048 + 512 * s:2048 + 512 * s + 512],
                                        sa[:], sb_[:], ALU.add)

            # ---- phase 4: out-proj (two passes) + residual + LN2 ----
            # cfb chunk s (s<4): features of heads 0-1 of source s;
            # chunk 4+s: features of heads 2-3 of source s.
            yb = rtile("yb", [128, 4096], HF)
            for eo in range(NEH):
                wof = wrk.tile([128, E], HF, tag="ws", name=f"wofA{eo}", bufs=3)
                nc.sync.dma_start(wof[:], wot_d[eo])
                po = pB(f"poA{eo}")
                for s in range(4):
                    nc.tensor.matmul(po[:], wof[:, 256 * s:256 * s + 128],
                                     cfb[:, 512 * s:512 * s + 512],
                                     start=(s == 0), stop=(s == 3))
                xrs = wrk.tile([128, 512], F32, tag="xrs", name=f"xrs{eo}", bufs=2)
                nc.sync.dma_start(xrs[:], xres_d[128 * eo:128 * eo + 128, :])
                nc.vector.scalar_tensor_tensor(
                    out=yb[:, 512 * eo:512 * eo + 512], in0=po[:],
                    scalar=bo[:, eo:eo + 1], in1=xrs[:],
                    op0=ALU.add, op1=ALU.add)
            pxq2 = pA("pxq2", (1, 1024))
            sq2_l = []
            for eo in range(NEH):
                wof = wrk.tile([128, E], HF, tag="ws", name=f"wofB{eo}", bufs=3)
                nc.sync.dma_start(wof[:], wot_d[eo])
                po = pB(f"poB{eo}")
                for s in range(4):
                    nc.tensor.matmul(po[:], wof[:, 256 * s + 128:256 * s + 256],
                                     cfb[:, 2048 + 512 * s:2048 + 512 * s + 512],
                                     start=(s == 0), stop=(s == 3))
                ybe = yb[:, 512 * eo:512 * eo + 512]
                nc.vector.tensor_tensor(ybe, po[:], ybe, ALU.add)
                # LN2 sums interleaved with out-proj pass B
                sq = wrk.tile([128, 512], HF, tag="sqh", name=f"sq2_{eo}",
                              bufs=3)
                nc.scalar.activation(sq[:], ybe, AF.Square)
                nc.tensor.matmul(pxq2[:, 512:1024], onch[:], sq[:],
                                 start=(eo == 0), stop=(eo == NEH - 1))
                nc.tensor.matmul(pxq2[:, 0:512], onch[:], ybe,
                                 start=(eo == 0), stop=(eo == NEH - 1))
                sq2_l.append(sq)
            # LN2 stats chain
            a_row2, c_row2 = ln_chain(pxq2, 512, "l2")
            bch2 = ln_bc(a_row2, c_row2, 512, "l2")
            x2r = rtile("x2rb", [128, 4096], HF, tag="slabW")
            for e in range(NEH):
                w0 = 512 * e
                tmp = wrk.tile([128, 512], HF, tag="lnt", name=f"lnt2_{e}", bufs=3)
                nc.vector.tensor_tensor(tmp[:], yb[:, w0:w0 + 512],
                                        bch2[:, 0:512], ALU.mult)
                tm2 = wrk.tile([128, 512], HF, tag="lnu", name=f"lnu2_{e}", bufs=3)
                nc.vector.tensor_tensor(tm2[:], tmp[:], bch2[:, 512:1024], ALU.add)
                nc.vector.tensor_scalar(
                    out=x2r[:, w0:w0 + 512], in0=tm2[:],
                    scalar1=g2c[:, e:e + 1], scalar2=b2c[:, e:e + 1],
                    op0=ALU.mult, op1=ALU.add)

            # ---- phase 5: FFN (fc1/fc2 interleaved by f-group) ----
            outT = rtile("outTb", [128, 4096], F32, tag="slabF")
            for grp in range(NFH // FG):
                fs = list(range(FG * grp, FG * (grp + 1)))
                h1_l = []
                for f in fs:
                    w1f = wrk.tile([128, E], HF, tag="ws", name=f"w1f{f}", bufs=3)
                    nc.sync.dma_start(w1f[:], w1_d[f])
                    pf = pA(f"pf{f}", (128, 512))
                    for e in range(NEH):
                        nc.tensor.matmul(pf[:], w1f[:, 128 * e:128 * e + 128],
                                         x2r[:, 512 * e:512 * e + 512],
                                         start=(e == 0), stop=(e == NEH - 1))
                    h1f = wrk.tile([128, 512], HF, tag="h1g", name=f"h1_{f}",
                                   bufs=FG + 2)
                    nc.scalar.activation(h1f[:], pf[:], AF.Gelu, bias=b1[:, f:f + 1])
                    h1_l.append(h1f)
                w2f_l = []
                for f in fs:
                    w2f = wrk.tile([128, E], HF, tag="w2s", name=f"w2f{f}",
                                   bufs=FG + 1)
                    nc.sync.dma_start(w2f[:], w2_d[128 * f:128 * f + 128, :])
                    w2f_l.append(w2f)
                for eo in range(NEH):
                    p2 = pB(f"p2_{grp}_{eo}")
                    for u in range(FG):
                        nc.tensor.matmul(p2[:], w2f_l[u][:, 128 * eo:128 * eo + 128],
                                         h1_l[u][:], start=(u == 0),
                                         stop=(u == FG - 1))
                    o_ = outT[:, 512 * eo:512 * eo + 512]
                    if grp == 0:
                        nc.vector.tensor_copy(o_, p2[:])
                    else:
                        nc.vector.tensor_tensor(o_, p2[:], o_, ALU.add)
                    if grp == NFH // FG - 1:
                        nc.vector.scalar_tensor_tensor(
                            out=o_, in0=o_, scalar=b2[:, eo:eo + 1],
                            in1=x2r[:, 512 * eo:512 * eo + 512],
                            op0=ALU.add, op1=ALU.add)

            # ---- phase 6: transpose to [tokens, E] and store ----
            for ts_ in range(TS // 128):
                osb = wrk.tile([128, E], F32, tag="osb", name=f"osb{ts_}", bufs=2)
                for eo in range(NEH):
                    pt = pB(f"pt{ts_}_{eo}", (128, 128))
                    nc.tensor.transpose(
                        pt[:],
                        outT[:, 512 * eo + 128 * ts_:512 * eo + 128 * ts_ + 128],
                        ident[:])
                    nc.vector.tensor_copy(osb[:, 128 * eo:128 * eo + 128], pt[:])
                nc.sync.dma_start(out_d[128 * ts_:128 * ts_ + 128, :], osb[:])

    nc.compile()
    return nc


_CACHE = {}


def _get_nc(causal):
    if causal not in _CACHE:
        _CACHE[causal] = build(causal)
    return _CACHE[causal]


def _build_in_maps(x, mask, Wq, bq, Wk, bk, Wv, bv, Wo, bo, W1, b1, W2, b2,
                   ln1_g, ln1_b, ln2_g, ln2_b):
    x = np.asarray(x, np.float32)
    mask2d = np.asarray(mask).reshape(S, S)
    causal = bool(np.array_equal(mask2d, np.tril(np.ones((S, S), mask2d.dtype))))

    def colmaj(v, nch):
        return np.ascontiguousarray(np.asarray(v, np.float32).reshape(nch, 128).T)

    def bf(a):
        return np.ascontiguousarray(np.asarray(a, np.float32).astype(BF_NP))

    ln1_g = np.asarray(ln1_g, np.float32)
    ln1_b = np.asarray(ln1_b, np.float32)
    Wq = np.asarray(Wq, np.float32)
    Wk = np.asarray(Wk, np.float32)
    Wv = np.asarray(Wv, np.float32)
    Wo = np.asarray(Wo, np.float32)
    W1 = np.asarray(W1, np.float32)
    Wqf = Wq * ln1_g[:, None] / np.sqrt(D)
    bqf = (ln1_b @ Wq + np.asarray(bq, np.float32)) / np.sqrt(D)
    Wkf = Wk * ln1_g[:, None]
    bkf = ln1_b @ Wk + np.asarray(bk, np.float32)
    Wvf = Wv * ln1_g[:, None]
    bvf = ln1_b @ Wv + np.asarray(bv, np.float32)
    w1t = np.ascontiguousarray(
        W1.reshape(NEH, 128, NFH, 128).transpose(2, 1, 0, 3).reshape(NFH, 128, E))
    # wot chunk s: rows = heads (2s%?) -- feature-major [contract 128, E out]
    wot = np.ascontiguousarray(
        Wo.reshape(NEH, 128, NEH, 128).transpose(2, 1, 0, 3).reshape(NEH, 128, E))

    cm = np.zeros((128, 4 * 512), np.float32)
    for j in range(4):
        p, q = np.meshgrid(np.arange(128), np.arange(512), indexing='ij')
        cm[:, 512 * j:512 * (j + 1)] = (q >= p + 128 * j).astype(np.float32)

    xT = [np.ascontiguousarray(x[b_].T) for b_ in range(B)]
    xTh = [bf(a) for a in xT]
    shared = {
        "wot": bf(wot), "bo": colmaj(bo, NEH),
        "w1": bf(w1t), "b1": colmaj(b1, NFH),
        "w2": bf(np.asarray(W2, np.float32)),
        "b2": colmaj(b2, NEH),
        "g2c": colmaj(ln2_g, NEH),
        "b2c": colmaj(ln2_b, NEH),
        "cmask": bf(cm),
        "onch": np.ones((128, 1), BF_NP),
        "onrr": np.ones((1, 128), np.float32),
        "onrh": np.ones((1, 128), BF_NP),
        "on4": np.ones((128, HPC), BF_NP),
    }
    if not causal:
        shared["mT"] = np.ascontiguousarray(
            np.where(mask2d == 0, np.float32(-1e9), np.float32(0.0)).T)
    zms = []
    for b_ in range(B):
        z_ = np.zeros((128, NC), np.float32)
        z_[:, 4 * b_:4 * b_ + 4] = 1.0
        zms.append(z_)

    in_maps = []
    for c in range(NC):
        b_, j = c // 4, c % 4
        cs = slice(HPC * D * j, HPC * D * (j + 1))
        m = {
            "xT": xTh[b_], "zm": zms[b_],
            "xres": np.ascontiguousarray(xT[b_][:, TS * j:TS * (j + 1)]),
            "wq": bf(Wqf[:, cs]),
            "wk": bf(Wkf[:, cs]),
            "wv": bf(Wvf[:, cs]),
            "bq": colmaj(bqf[cs], MCH),
            "bk": colmaj(bkf[cs], MCH),
            "bv": bf(bvf[None, cs]),
        }
        m.update(shared)
        in_maps.append(m)
    return in_maps, causal


def kernel(**inputs):
    in_maps, causal = _build_in_maps(**inputs)
    nc_obj = _get_nc(causal)
    res = run_bass_kernel_spmd(nc_obj, in_maps, list(range(NC)))
    out = np.empty((B, S, E), np.float32)
    for c in range(NC):
        b_, j = c // 4, c % 4
        out[b_, TS * j:TS * (j + 1), :] = res.results[c]["out"]
    return out


# revision 37
# speedup vs baseline: 1.1014x; 1.1014x over previous
"""Decoder block (LN1 -> causal MHA -> LN2 -> GELU FFN, residuals) on 8 NeuronCores.

Sharding: 2-way data parallel over batch x 4-way tensor parallel over heads.
Core c: batch b=c//4, heads [4*(c%4) .. 4*(c%4)+4); after an AllToAll of
per-head attention context, core c owns token slice [512*(c%4) .. +512) of its
batch for out-proj / LN2 / FFN.

Activations live in transposed layout [feature, token]; all large matmuls run
in bfloat16 (fast weight load, fp32 PSUM accumulate).  The AllToAll is split
in two (heads 0-1 / heads 2-3): the first overlaps the second half of
attention, the second overlaps the first out-projection pass.

SBUF slab tags shared across phases:
  slabW [128, 4096] bf16: wk+wv (phase 1)  ->  x2r (phases 4-6)
  slabE [128, 4160] bf16: V'   (phases 1-3) ->  cf (phase 4+)
"""
import sys
import numpy as np

sys.path.insert(0, '/opt/trn_rl_repo')

import ml_dtypes                       # noqa: E402
import concourse.bass as bass          # noqa: E402
import concourse.bacc as bacc          # noqa: E402
import concourse.tile as tile          # noqa: E402
from concourse import mybir            # noqa: E402
from concourse.masks import make_identity  # noqa: E402
from concourse.bass_utils import run_bass_kernel_spmd  # noqa: E402

F32 = mybir.dt.float32
F32R = mybir.dt.float32r
HF = mybir.dt.bfloat16
BF_NP = ml_dtypes.bfloat16
AF = mybir.ActivationFunctionType
ALU = mybir.AluOpType

B, S, E, H, D, F = 2, 2048, 1024, 16, 64, 4096
NC = 8
T = S
TS = 512
EPS = 1e-5
NEH = E // 128         # 8
NFH = F // 128         # 32
HPC = 4                # heads per core
MCH = 2                # d-chunks for 4 heads
LW = 512               # layernorm / projection chunk width
NCH = T // LW          # 4 token chunks
FG = 8                 # fc1/fc2 f-chunks per interleaved group


def build(causal=True):
    nc = bacc.Bacc("TRN2", target_bir_lowering=False, debug=False, num_devices=NC)

    xT_d = nc.dram_tensor("xT", [E, T], HF, kind="ExternalInput").ap()
    wq_d = nc.dram_tensor("wq", [E, HPC * D], HF, kind="ExternalInput").ap()
    wk_d = nc.dram_tensor("wk", [E, HPC * D], HF, kind="ExternalInput").ap()
    wv_d = nc.dram_tensor("wv", [E, HPC * D], HF, kind="ExternalInput").ap()
    bq_d = nc.dram_tensor("bq", [128, MCH], F32, kind="ExternalInput").ap()
    bk_d = nc.dram_tensor("bk", [128, MCH], F32, kind="ExternalInput").ap()
    bv_d = nc.dram_tensor("bv", [1, HPC * D], HF, kind="ExternalInput").ap()
    wot_d = nc.dram_tensor("wot", [NEH, 128, E], HF, kind="ExternalInput").ap()
    bo_d = nc.dram_tensor("bo", [128, NEH], F32, kind="ExternalInput").ap()
    w1_d = nc.dram_tensor("w1", [NFH, 128, E], HF, kind="ExternalInput").ap()
    b1_d = nc.dram_tensor("b1", [128, NFH], F32, kind="ExternalInput").ap()
    w2_d = nc.dram_tensor("w2", [F, E], HF, kind="ExternalInput").ap()
    b2_d = nc.dram_tensor("b2", [128, NEH], F32, kind="ExternalInput").ap()
    g2c_d = nc.dram_tensor("g2c", [128, NEH], F32, kind="ExternalInput").ap()
    b2c_d = nc.dram_tensor("b2c", [128, NEH], F32, kind="ExternalInput").ap()
    xres_d = nc.dram_tensor("xres", [E, TS], F32, kind="ExternalInput").ap()
    zm_d = nc.dram_tensor("zm", [128, NC], F32, kind="ExternalInput").ap()
    cmask_d = nc.dram_tensor("cmask", [128, 4 * 512], HF, kind="ExternalInput").ap()
    onch_d = nc.dram_tensor("onch", [128, 1], HF, kind="ExternalInput").ap()
    onrr_d = nc.dram_tensor("onrr", [1, 128], F32R, kind="ExternalInput").ap()
    onrh_d = nc.dram_tensor("onrh", [1, 128], HF, kind="ExternalInput").ap()
    on4_d = nc.dram_tensor("on4", [128, HPC], HF, kind="ExternalInput").ap()
    if not causal:
        mT_d = nc.dram_tensor("mT", [S, S], F32, kind="ExternalInput").ap()
    out_d = nc.dram_tensor("out", [TS, E], F32, kind="ExternalOutput").ap()

    with tile.TileContext(nc) as tc:
        with (
            tc.tile_pool(name="res", bufs=1) as res,
            tc.tile_pool(name="wk_", bufs=2) as wrk,
            tc.tile_pool(name="st", bufs=1) as st,
            tc.tile_pool(name="ps", bufs=2, space="PSUM") as ps,
            tc.tile_pool(name="dram", bufs=1, space="DRAM") as dram,
        ):
            def rtile(name, shape, dt, tag=None):
                return res.tile(shape, dt, tag=tag or name, name=name)

            def pA(name, shape=(128, 1024)):
                return ps.tile(list(shape), F32, tag="pA", name=name, bufs=3)

            def pB(name, shape=(128, 512)):
                return ps.tile(list(shape), F32, tag="pB", name=name, bufs=2)

            # ---- constants ----
            onch = rtile("onch", [128, 1], HF)
            nc.sync.dma_start(onch[:], onch_d[:])
            onrr = rtile("onrr", [1, 128], F32R)
            nc.sync.dma_start(onrr[:], onrr_d[:])
            onrh = rtile("onrh", [1, 128], HF)
            nc.sync.dma_start(onrh[:], onrh_d[:])
            on4 = rtile("on4", [128, HPC], HF)
            nc.sync.dma_start(on4[:], on4_d[:])
            ident = rtile("ident", [128, 128], F32)
            make_identity(nc, ident[:])
            zm = rtile("zm", [128, NC], F32)
            nc.sync.dma_start(zm[:], zm_d[:])
            cmask = rtile("cmask", [128, 4 * 512], HF)
            nc.sync.dma_start(cmask[:], cmask_d[:])
            bq = rtile("bq", [128, MCH], F32)
            nc.sync.dma_start(bq[:], bq_d[:])
            bk = rtile("bk", [128, MCH], F32)
            nc.sync.dma_start(bk[:], bk_d[:])
            bv = rtile("bv", [1, HPC * D], HF)
            nc.sync.dma_start(bv[:], bv_d[:])
            bo = rtile("bo", [128, NEH], F32)
            nc.sync.dma_start(bo[:], bo_d[:])
            b1 = rtile("b1", [128, NFH], F32)
            nc.sync.dma_start(b1[:], b1_d[:])
            b2 = rtile("b2", [128, NEH], F32)
            nc.sync.dma_start(b2[:], b2_d[:])
            g2c = rtile("g2c", [128, NEH], F32)
            nc.sync.dma_start(g2c[:], g2c_d[:])
            b2c = rtile("b2c", [128, NEH], F32)
            nc.sync.dma_start(b2c[:], b2c_d[:])

            # ---- resident weights/tensors ----
            wqb = rtile("wqb", [128, NEH * HPC * D], HF)        # [128, 2048]
            for e in range(NEH):
                nc.sync.dma_start(wqb[:, 256 * e:256 * e + 256],
                                  wq_d[128 * e:128 * e + 128, :])
            wkvb = rtile("wkvb", [128, 4096], HF, tag="slabW")
            for e in range(NEH):
                nc.sync.dma_start(wkvb[:, 256 * e:256 * e + 256],
                                  wk_d[128 * e:128 * e + 128, :])
                nc.sync.dma_start(wkvb[:, 2048 + 256 * e:2048 + 256 * e + 256],
                                  wv_d[128 * e:128 * e + 128, :])
            qtb = rtile("qtb", [128, MCH * T], HF)              # [128, 4096]
            ktb = rtile("ktb", [128, MCH * T], HF)              # [128, 4096]
            vpb = rtile("vpb", [128, (T // 128) * HPC * 65], HF, tag="slabE")
            cfb = rtile("cfb", [128, 4096], HF)

            def vsl(tt):
                return vpb[:, 260 * tt:260 * tt + 260]

            # ---- layernorm helpers ----
            def ln_chain(pxq, width, idx):
                """DVE stats chain on accumulated [1,1024] (sum|sumsq) PSUM tile;
                returns (a_row, c_row) [1,width] f32r tiles (rstd, -mu*rstd)."""
                mean = st.tile([1, width], F32, tag="mean", name=f"mean{idx}")
                nc.vector.tensor_scalar_mul(mean[:], pxq[:, 0:width], 1.0 / E)
                msq = st.tile([1, width], F32, tag="msq", name=f"msq{idx}")
                nc.vector.tensor_scalar_mul(msq[:], pxq[:, 512:512 + width], 1.0 / E)
                scr = st.tile([1, width], F32, tag="scr", name=f"scr{idx}")
                nc.vector.tensor_tensor(scr[:], mean[:], mean[:], ALU.mult)
                var = st.tile([1, width], F32, tag="var", name=f"var{idx}")
                nc.vector.scalar_tensor_tensor(
                    out=var[:], in0=msq[:], scalar=float(EPS), in1=scr[:],
                    op0=ALU.add, op1=ALU.subtract)
                std = st.tile([1, width], F32, tag="std", name=f"std{idx}")
                nc.scalar.activation(std[:], var[:], AF.Sqrt)
                rstd = st.tile([1, width], F32, tag="rstd", name=f"rstd{idx}")
                nc.vector.reciprocal_approx_fast(rstd[:], std[:])
                a_row = st.tile([1, width], F32R, tag="a_row", name=f"a_row{idx}")
                nc.vector.tensor_copy(a_row[:], rstd[:])
                ct = st.tile([1, width], F32, tag="ct", name=f"ct{idx}")
                nc.vector.tensor_tensor(ct[:], mean[:], rstd[:], ALU.mult)
                c_row = st.tile([1, width], F32R, tag="c_row", name=f"c_row{idx}")
                nc.vector.tensor_scalar_mul(c_row[:], ct[:], -1.0)
                return a_row, c_row

            def ln_bc(a_row, c_row, width, idx):
                """Broadcast a/c rows into a bf16 SBUF tile [128, 1024]:
                [:, :512]=a, [:, 512:]=c."""
                bc = pA(f"bc{idx}")
                nc.tensor.matmul(bc[:, 0:width], onrr[:], a_row[:],
                                 start=True, stop=True)
                nc.tensor.matmul(bc[:, 512:512 + width], onrr[:], c_row[:],
                                 start=True, stop=True)
                bch = wrk.tile([128, 1024], HF, tag="bch", name=f"bch{idx}", bufs=2)
                nc.scalar.copy(bch[:, 0:width], bc[:, 0:width])
                nc.scalar.copy(bch[:, 512:512 + width], bc[:, 512:512 + width])
                return bch

            # ---- phase 1: LN1 fused with Q/K/V projections, per 512 tokens ----
            # Chunk pipeline: chunk t's zz (DVE) production is interleaved
            # per-feature-chunk with its m=0 projections and chunk t+1's LN
            # sums on PE, so neither engine waits on the other's full chunk.
            xt_all, zz_all, pxq_all, row_all = [], [], [], []

            def p1_load(tcn):
                xtc = wrk.tile([128, NEH * LW], HF, tag="xt", name=f"xt{tcn}",
                               bufs=3)
                for u in range(2):
                    nc.sync.dma_start(
                        xtc[:, 2048 * u:2048 * u + 2048]
                        .rearrange("p (e w) -> p e w", e=4),
                        xT_d[512 * u:512 * u + 512, LW * tcn:LW * tcn + LW]
                        .rearrange("(e p) w -> p e w", p=128))
                xt_all.append(xtc)

            def p1_sums_e(tcn, e):
                if e == 0:
                    pxq_all.append(pA(f"pxq{tcn}", (1, 1024)))
                pxq = pxq_all[tcn]
                src = xt_all[tcn][:, 512 * e:512 * e + 512]
                sq = wrk.tile([128, LW], HF, tag="sqh", name=f"sq{tcn}_{e}",
                              bufs=3)
                nc.scalar.activation(sq[:], src, AF.Square)
                nc.tensor.matmul(pxq[:, 512:1024], onch[:], sq[:],
                                 start=(e == 0), stop=(e == NEH - 1))
                nc.tensor.matmul(pxq[:, 0:512], onch[:], src,
                                 start=(e == 0), stop=(e == NEH - 1))

            def p1_chain(tcn):
                rows = ln_chain(pxq_all[tcn], LW, f"l1c{tcn}")
                row_all.append(rows)

            def p1_zz_e(tcn, e, bch):
                tmp = wrk.tile([128, LW], HF, tag="lnt", name=f"lnt{tcn}_{e}",
                               bufs=2)
                nc.vector.tensor_tensor(tmp[:], xt_all[tcn][:, 512 * e:512 * e + 512],
                                        bch[:, 0:LW], ALU.mult)
                zz = wrk.tile([128, LW], HF, tag="zz", name=f"zz{tcn}_{e}",
                              bufs=9)
                nc.vector.tensor_tensor(zz[:], tmp[:], bch[:, 512:512 + LW],
                                        ALU.add)
                zz_all[tcn].append(zz)

            p1_load(0)
            p1_load(1)
            for e in range(NEH):
                p1_sums_e(0, e)
            p1_chain(0)
            for tcn in range(NCH):
                if tcn + 2 < NCH:
                    p1_load(tcn + 2)
                a_row, c_row = row_all[tcn]
                bch = ln_bc(a_row, c_row, LW, f"l1c{tcn}")
                zz_all.append([])
                zz_l = zz_all[tcn]
                pq0 = pB(f"pq{tcn}_0", (128, LW))
                pk0 = pB(f"pk{tcn}_0", (128, LW))
                for e in range(NEH):
                    p1_zz_e(tcn, e, bch)
                    nc.tensor.matmul(pq0[:], wqb[:, 256 * e:256 * e + 128],
                                     zz_l[e][:], start=(e == 0), stop=(e == NEH - 1))
                    nc.tensor.matmul(pk0[:], wkvb[:, 256 * e:256 * e + 128],
                                     zz_l[e][:], start=(e == 0), stop=(e == NEH - 1))
                    if tcn + 1 < NCH:
                        p1_sums_e(tcn + 1, e)
                c0 = LW * tcn
                nc.vector.tensor_scalar_add(qtb[:, c0:c0 + LW], pq0[:],
                                            bq[:, 0:1])
                nc.vector.tensor_scalar_add(ktb[:, c0:c0 + LW], pk0[:],
                                            bk[:, 0:1])
                pq1 = pB(f"pq{tcn}_1", (128, LW))
                for e in range(NEH):
                    nc.tensor.matmul(pq1[:], wqb[:, 256 * e + 128:256 * e + 256],
                                     zz_l[e][:], start=(e == 0), stop=(e == NEH - 1))
                pk1 = pB(f"pk{tcn}_1", (128, LW))
                for e in range(NEH):
                    nc.tensor.matmul(pk1[:], wkvb[:, 256 * e + 128:256 * e + 256],
                                     zz_l[e][:], start=(e == 0), stop=(e == NEH - 1))
                c1 = 2048 + LW * tcn
                nc.vector.tensor_scalar_add(qtb[:, c1:c1 + LW], pq1[:],
                                            bq[:, 1:2])
                nc.vector.tensor_scalar_add(ktb[:, c1:c1 + LW], pk1[:],
                                            bk[:, 1:2])
                for sub in range(LW // 128):
                    tt = (LW * tcn) // 128 + sub
                    pv = pB(f"pv{tt}", (128, HPC * D))
                    for e in range(NEH):
                        nc.tensor.matmul(
                            pv[:], zz_l[e][:, 128 * sub:128 * sub + 128],
                            wkvb[:, 2048 + 256 * e:2048 + 256 * e + 256],
                            start=(e == 0), stop=False)
                    nc.tensor.matmul(pv[:], onrh[:], bv[:], start=False, stop=True)
                    nc.vector.tensor_copy(
                        vsl(tt).rearrange("p (h d) -> p h d", h=HPC)[:, :, 0:D],
                        pv[:].rearrange("p (h d) -> p h d", h=HPC))
                    nc.vector.tensor_copy(
                        vsl(tt).rearrange("p (h d) -> p h d", h=HPC)[:, :, D:D + 1],
                        on4[:].rearrange("p (h o) -> p h o", o=1))
                if tcn + 1 < NCH:
                    p1_chain(tcn + 1)

            # ---- phase 3: attention (h-major; A2A #1 after heads 0-1) ----
            stageA = dram.tile([NC, 2 * D, 512], HF, tag="stageA", name="stageA")
            stageB = dram.tile([NC, 2 * D, 512], HF, tag="stageB", name="stageB")
            stage2A = dram.tile([NC, 2 * D, 512], HF, tag="stage2A", name="stage2A")
            stage2B = dram.tile([NC, 2 * D, 512], HF, tag="stage2B", name="stage2B")
            NSB = T // 512
            saA_l = []
            for h in range(HPC):
                for i in range(NSB):
                    m, hp = h // 2, h % 2
                    qsl = qtb[64 * hp:64 * hp + 64, 2048 * m + 512 * i:
                              2048 * m + 512 * i + 512]
                    n_kc = 4 * (i + 1) if causal else T // 128
                    psav = pB(f"av{h}_{i}")
                    ngr = (n_kc + 1) // 2
                    psc_l = {}

                    def emit_qk(g, h=h, i=i, m=m, hp=hp, qsl=qsl, n_kc=n_kc,
                                psc_l=psc_l):
                        kcs = [k for k in (2 * g, 2 * g + 1) if k < n_kc]
                        psc = pA(f"sc{h}_{i}_{g}")
                        for u, kc in enumerate(kcs):
                            nc.tensor.matmul(
                                psc[:, 512 * u:512 * u + 512],
                                ktb[64 * hp:64 * hp + 64,
                                    2048 * m + 128 * kc:2048 * m + 128 * kc + 128],
                                qsl, start=True, stop=True)
                        if not causal:
                            for u, kc in enumerate(kcs):
                                mb = wrk.tile([128, 512], F32, tag="mb",
                                              name=f"mb{h}_{i}_{g}_{u}", bufs=2)
                                nc.sync.dma_start(
                                    mb[:], mT_d[128 * kc:128 * kc + 128,
                                                512 * i:512 * i + 512])
                                nc.vector.tensor_tensor(
                                    psc[:, 512 * u:512 * u + 512],
                                    psc[:, 512 * u:512 * u + 512], mb[:], ALU.add)
                        psc_l[g] = (psc, kcs)

                    emit_qk(0)
                    for g in range(ngr):
                        if g + 1 < ngr:
                            emit_qk(g + 1)
                        psc, kcs = psc_l.pop(g)
                        eg = wrk.tile([128, 1024], HF, tag="exp",
                                      name=f"exp{h}_{i}_{g}", bufs=3)
                        w = 512 * len(kcs)
                        nc.scalar.activation(eg[:, 0:w], psc[:, 0:w], AF.Exp)
                        if causal:
                            for u, kc in enumerate(kcs):
                                j = kc - 4 * i
                                if j >= 0:
                                    nc.vector.tensor_tensor(
                                        eg[:, 512 * u:512 * u + 512],
                                        eg[:, 512 * u:512 * u + 512],
                                        cmask[:, 512 * j:512 * j + 512], ALU.mult)
                        for u, kc in enumerate(kcs):
                            nc.tensor.matmul(
                                psav[0:65, :], vsl(kc)[:, 65 * h:65 * h + 65],
                                eg[:, 512 * u:512 * u + 512],
                                start=(kc == 0), stop=(kc == n_kc - 1),
                                skip_group_check=True)
                    sden = st.tile([1, 512], F32, tag="sden", name=f"sden{h}_{i}")
                    nc.scalar.copy(sden[:], psav[64:65, :])
                    rc = st.tile([1, 512], F32, tag="rc", name=f"rc{h}_{i}")
                    nc.vector.reciprocal_approx_fast(rc[:], sden[:])
                    rch = st.tile([1, 512], HF, tag="rch", name=f"rch{h}_{i}")
                    nc.vector.tensor_copy(rch[:], rc[:])
                    rb = wrk.tile([64, 512], HF, tag="rb", name=f"rb{h}_{i}", bufs=2)
                    nc.gpsimd.partition_broadcast(rb[:], rch[:])
                    ctx = wrk.tile([64, 512], HF, tag="ctx", name=f"ctx{h}_{i}",
                                   bufs=3)
                    nc.vector.tensor_tensor(ctx[:], psav[0:64, :], rb[:], ALU.mult)
                    stg = stageA if h < 2 else stageB
                    hh = h % 2
                    for s_ in (i, i + 4):
                        cz = wrk.tile([64, 512], HF, tag="cz",
                                      name=f"cz{h}_{i}_{s_}", bufs=3)
                        nc.vector.tensor_scalar_mul(cz[:], ctx[:],
                                                    zm[0:64, s_:s_ + 1])
                        nc.sync.dma_start(stg[s_][64 * hh:64 * hh + 64, :], cz[:])
                if h == 1:
                    nc.gpsimd.collective_compute(
                        "AllToAll", ALU.bypass, replica_groups=[list(range(NC))],
                        ins=[stageA.opt()], outs=[stage2A.opt()])
                if h == 2:
                    # pull the A2A#1 results in on the gpsimd queue (DMA only
                    # -- no gpsimd compute, which would thrash the Q7
                    # microcode library against partition_broadcast) while
                    # head 3's attention still runs on PE/ACT/DVE/Sync
                    for s in range(4):
                        sa = wrk.tile([128, 512], HF, tag="sa", name=f"saA{s}",
                                      bufs=8)
                        nc.gpsimd.dma_start(sa[:], stage2A[s])
                        sb_ = wrk.tile([128, 512], HF, tag="sa", name=f"sbA{s}",
                                       bufs=8)
                        nc.gpsimd.dma_start(sb_[:], stage2A[4 + s])
                        saA_l.append((sa, sb_))
            nc.gpsimd.collective_compute(
                "AllToAll", ALU.bypass, replica_groups=[list(range(NC))],
                ins=[stageB.opt()], outs=[stage2B.opt()])
            saB_l = []
            for s in range(4):
                sa = wrk.tile([128, 512], HF, tag="sa", name=f"saB{s}", bufs=8)
                nc.gpsimd.dma_start(sa[:], stage2B[s])
                sb_ = wrk.tile([128, 512], HF, tag="sa", name=f"sbB{s}", bufs=8)
                nc.gpsimd.dma_start(sb_[:], stage2B[4 + s])
                saB_l.append((sa, sb_))

            # ---- phase 4: out-proj (two passes) + residual + LN2 ----
            # cfb chunk s (s<4): features of heads 0-1 of source s;
            # chunk 4+s: features of heads 2-3 of source s.
            yb = rtile("yb", [128, 4096], HF)
            for s in range(4):
                sa, sb_ = saA_l[s]
                nc.vector.tensor_tensor(cfb[:, 512 * s:512 * s + 512],
                                        sa[:], sb_[:], ALU.add)
            for eo in range(NEH):
                wof = wrk.tile([128, E], HF, tag="ws", name=f"wofA{eo}", bufs=3)
                nc.sync.dma_start(wof[:], wot_d[eo])
                po = pB(f"poA{eo}")
                for s in range(4):
                    nc.tensor.matmul(po[:], wof[:, 256 * s:256 * s + 128],
                                     cfb[:, 512 * s:512 * s + 512],
                                     start=(s == 0), stop=(s == 3))
                xrs = wrk.tile([128, 512], F32, tag="xrs", name=f"xrs{eo}", bufs=2)
                nc.sync.dma_start(xrs[:], xres_d[128 * eo:128 * eo + 128, :])
                nc.vector.scalar_tensor_tensor(
                    out=yb[:, 512 * eo:512 * eo + 512], in0=po[:],
                    scalar=bo[:, eo:eo + 1], in1=xrs[:],
                    op0=ALU.add, op1=ALU.add)
            for s in range(4):
                sa, sb_ = saB_l[s]
                nc.vector.tensor_tensor(cfb[:, 2048 + 512 * s:2048 + 512 * s + 512],
                                        sa[:], sb_[:], ALU.add)
            pxq2 = pA("pxq2", (1, 1024))
            sq2_l = []
            for eo in range(NEH):
                wof = wrk.tile([128, E], HF, tag="ws", name=f"wofB{eo}", bufs=3)
                nc.sync.dma_start(wof[:], wot_d[eo])
                po = pB(f"poB{eo}")
                for s in range(4):
                    nc.tensor.matmul(po[:], wof[:, 256 * s + 128:256 * s + 256],
                                     cfb[:, 2048 + 512 * s:2048 + 512 * s + 512],
                                     start=(s == 0), stop=(s == 3))
                ybe = yb[:, 512 * eo:512 * eo + 512]
                nc.vector.tensor_tensor(ybe, po[:], ybe, ALU.add)
                # LN2 sums interleaved with out-proj pass B
                sq = wrk.tile([128, 512], HF, tag="sqh", name=f"sq2_{eo}",
                              bufs=3)
                nc.scalar.activation(sq[:], ybe, AF.Square)
                nc.tensor.matmul(pxq2[:, 512:1024], onch[:], sq[:],
                                 start=(eo == 0), stop=(eo == NEH - 1))
                nc.tensor.matmul(pxq2[:, 0:512], onch[:], ybe,
                                 start=(eo == 0), stop=(eo == NEH - 1))
                sq2_l.append(sq)
            # LN2 stats chain
            a_row2, c_row2 = ln_chain(pxq2, 512, "l2")
            bch2 = ln_bc(a_row2, c_row2, 512, "l2")
            x2r = rtile("x2rb", [128, 4096], HF, tag="slabW")
            for e in range(NEH):
                w0 = 512 * e
                tmp = wrk.tile([128, 512], HF, tag="lnt", name=f"lnt2_{e}", bufs=2)
                nc.vector.tensor_tensor(tmp[:], yb[:, w0:w0 + 512],
                                        bch2[:, 0:512], ALU.mult)
                tm2 = wrk.tile([128, 512], HF, tag="lnu", name=f"lnu2_{e}", bufs=2)
                nc.vector.tensor_tensor(tm2[:], tmp[:], bch2[:, 512:1024], ALU.add)
                nc.vector.tensor_scalar(
                    out=x2r[:, w0:w0 + 512], in0=tm2[:],
                    scalar1=g2c[:, e:e + 1], scalar2=b2c[:, e:e + 1],
                    op0=ALU.mult, op1=ALU.add)

            # ---- phase 5: FFN (fc1/fc2 interleaved by f-group) ----
            outT = rtile("outTb", [128, 4096], F32, tag="slabF")
            for grp in range(NFH // FG):
                fs = list(range(FG * grp, FG * (grp + 1)))
                h1_l = []
                for f in fs:
                    w1f = wrk.tile([128, E], HF, tag="ws", name=f"w1f{f}", bufs=3)
                    nc.sync.dma_start(w1f[:], w1_d[f])
                    pf = pA(f"pf{f}", (128, 512))
                    for e in range(NEH):
                        nc.tensor.matmul(pf[:], w1f[:, 128 * e:128 * e + 128],
                                         x2r[:, 512 * e:512 * e + 512],
                                         start=(e == 0), stop=(e == NEH - 1))
                    h1f = wrk.tile([128, 512], HF, tag="h1g", name=f"h1_{f}",
                                   bufs=FG + 2)
                    nc.scalar.activation(h1f[:], pf[:], AF.Gelu, bias=b1[:, f:f + 1])
                    h1_l.append(h1f)
                w2f_l = []
                for f in fs:
                    w2f = wrk.tile([128, E], HF, tag="w2s", name=f"w2f{f}",
                                   bufs=FG)
                    nc.sync.dma_start(w2f[:], w2_d[128 * f:128 * f + 128, :])
                    w2f_l.append(w2f)
                for eo in range(NEH):
                    p2 = pB(f"p2_{grp}_{eo}")
                    for u in range(FG):
                        nc.tensor.matmul(p2[:], w2f_l[u][:, 128 * eo:128 * eo + 128],
                                         h1_l[u][:], start=(u == 0),
                                         stop=(u == FG - 1))
                    o_ = outT[:, 512 * eo:512 * eo + 512]
                    if grp == 0:
                        nc.vector.tensor_copy(o_, p2[:])
                    else:
                        nc.vector.tensor_tensor(o_, p2[:], o_, ALU.add)
                    if grp == NFH // FG - 1:
                        nc.vector.scalar_tensor_tensor(
                            out=o_, in0=o_, scalar=b2[:, eo:eo + 1],
                            in1=x2r[:, 512 * eo:512 * eo + 512],
                            op0=ALU.add, op1=ALU.add)

            # ---- phase 6: transpose to [tokens, E] and store ----
            for ts_ in range(TS // 128):
                osb = wrk.tile([128, E], F32, tag="osb", name=f"osb{ts_}", bufs=2)
                for eo in range(NEH):
                    pt = pB(f"pt{ts_}_{eo}", (128, 128))
                    nc.tensor.transpose(
                        pt[:],
                        outT[:, 512 * eo + 128 * ts_:512 * eo + 128 * ts_ + 128],
                        ident[:])
                    nc.vector.tensor_copy(osb[:, 128 * eo:128 * eo + 128], pt[:])
                nc.sync.dma_start(out_d[128 * ts_:128 * ts_ + 128, :], osb[:])

    nc.compile()
    return nc


_CACHE = {}


def _get_nc(causal):
    if causal not in _CACHE:
        _CACHE[causal] = build(causal)
    return _CACHE[causal]


def _build_in_maps(x, mask, Wq, bq, Wk, bk, Wv, bv, Wo, bo, W1, b1, W2, b2,
                   ln1_g, ln1_b, ln2_g, ln2_b):
    x = np.asarray(x, np.float32)
    mask2d = np.asarray(mask).reshape(S, S)
    causal = bool(np.array_equal(mask2d, np.tril(np.ones((S, S), mask2d.dtype))))

    def colmaj(v, nch):
        return np.ascontiguousarray(np.asarray(v, np.float32).reshape(nch, 128).T)

    def bf(a):
        return np.ascontiguousarray(np.asarray(a, np.float32).astype(BF_NP))

    ln1_g = np.asarray(ln1_g, np.float32)
    ln1_b = np.asarray(ln1_b, np.float32)
    Wq = np.asarray(Wq, np.float32)
    Wk = np.asarray(Wk, np.float32)
    Wv = np.asarray(Wv, np.float32)
    Wo = np.asarray(Wo, np.float32)
    W1 = np.asarray(W1, np.float32)
    Wqf = Wq * ln1_g[:, None] / np.sqrt(D)
    bqf = (ln1_b @ Wq + np.asarray(bq, np.float32)) / np.sqrt(D)
    Wkf = Wk * ln1_g[:, None]
    bkf = ln1_b @ Wk + np.asarray(bk, np.float32)
    Wvf = Wv * ln1_g[:, None]
    bvf = ln1_b @ Wv + np.asarray(bv, np.float32)
    w1t = np.ascontiguousarray(
        W1.reshape(NEH, 128, NFH, 128).transpose(2, 1, 0, 3).reshape(NFH, 128, E))
    # wot chunk s: rows = heads (2s%?) -- feature-major [contract 128, E out]
    wot = np.ascontiguousarray(
        Wo.reshape(NEH, 128, NEH, 128).transpose(2, 1, 0, 3).reshape(NEH, 128, E))

    cm = np.zeros((128, 4 * 512), np.float32)
    for j in range(4):
        p, q = np.meshgrid(np.arange(128), np.arange(512), indexing='ij')
        cm[:, 512 * j:512 * (j + 1)] = (q >= p + 128 * j).astype(np.float32)

    xT = [np.ascontiguousarray(x[b_].T) for b_ in range(B)]
    xTh = [bf(a) for a in xT]
    shared = {
        "wot": bf(wot), "bo": colmaj(bo, NEH),
        "w1": bf(w1t), "b1": colmaj(b1, NFH),
        "w2": bf(np.asarray(W2, np.float32)),
        "b2": colmaj(b2, NEH),
        "g2c": colmaj(ln2_g, NEH),
        "b2c": colmaj(ln2_b, NEH),
        "cmask": bf(cm),
        "onch": np.ones((128, 1), BF_NP),
        "onrr": np.ones((1, 128), np.float32),
        "onrh": np.ones((1, 128), BF_NP),
        "on4": np.ones((128, HPC), BF_NP),
    }
    if not causal:
        shared["mT"] = np.ascontiguousarray(
            np.where(mask2d == 0, np.float32(-1e9), np.float32(0.0)).T)
    zms = []
    for b_ in range(B):
        z_ = np.zeros((128, NC), np.float32)
        z_[:, 4 * b_:4 * b_ + 4] = 1.0
        zms.append(z_)

    in_maps = []
    for c in range(NC):
        b_, j = c // 4, c % 4
        cs = slice(HPC * D * j, HPC * D * (j + 1))
        m = {
            "xT": xTh[b_], "zm": zms[b_],
            "xres": np.ascontiguousarray(xT[b_][:, TS * j:TS * (j + 1)]),
            "wq": bf(Wqf[:, cs]),
            "wk": bf(Wkf[:, cs]),
            "wv": bf(Wvf[:, cs]),
            "bq": colmaj(bqf[cs], MCH),
            "bk": colmaj(bkf[cs], MCH),
            "bv": bf(bvf[None, cs]),
        }
        m.update(shared)
        in_maps.append(m)
    return in_maps, causal


def kernel(**inputs):
    in_maps, causal = _build_in_maps(**inputs)
    nc_obj = _get_nc(causal)
    res = run_bass_kernel_spmd(nc_obj, in_maps, list(range(NC)))
    out = np.empty((B, S, E), np.float32)
    for c in range(NC):
        b_, j = c // 4, c % 4
        out[b_, TS * j:TS * (j + 1), :] = res.results[c]["out"]
    return out


# revision 42
# speedup vs baseline: 1.1765x; 1.0682x over previous
"""Decoder block (LN1 -> causal MHA -> LN2 -> GELU FFN, residuals) on 8 NeuronCores.

Sharding: 2-way data parallel over batch x 4-way tensor parallel over heads.
Core c: batch b=c//4, heads [4*(c%4) .. 4*(c%4)+4); after an AllToAll of
per-head attention context, core c owns token slice [512*(c%4) .. +512) of its
batch for out-proj / LN2 / FFN.

Activations live in transposed layout [feature, token]; all large matmuls run
in bfloat16 (fast weight load, fp32 PSUM accumulate).  The AllToAll is split
in two (heads 0-1 / heads 2-3): the first overlaps the second half of
attention, the second overlaps the first out-projection pass.

SBUF slab tags shared across phases:
  slabW [128, 4096] bf16: wk+wv (phase 1)  ->  x2r (phases 4-6)
  slabE [128, 4160] bf16: V'   (phases 1-3) ->  cf (phase 4+)
"""
import sys
import numpy as np

sys.path.insert(0, '/opt/trn_rl_repo')

import ml_dtypes                       # noqa: E402
import concourse.bass as bass          # noqa: E402
import concourse.bacc as bacc          # noqa: E402
import concourse.tile as tile          # noqa: E402
from concourse import mybir            # noqa: E402
from concourse.masks import make_identity  # noqa: E402
from concourse.bass_utils import run_bass_kernel_spmd  # noqa: E402

F32 = mybir.dt.float32
F32R = mybir.dt.float32r
HF = mybir.dt.bfloat16
BF_NP = ml_dtypes.bfloat16
AF = mybir.ActivationFunctionType
ALU = mybir.AluOpType

B, S, E, H, D, F = 2, 2048, 1024, 16, 64, 4096
NC = 8
T = S
TS = 512
EPS = 1e-5
NEH = E // 128         # 8
NFH = F // 128         # 32
HPC = 4                # heads per core
MCH = 2                # d-chunks for 4 heads
LW = 512               # layernorm / projection chunk width
NCH = T // LW          # 4 token chunks
FG = 8                 # fc1/fc2 f-chunks per interleaved group


def build(causal=True):
    nc = bacc.Bacc("TRN2", target_bir_lowering=False, debug=False, num_devices=NC)

    xT_d = nc.dram_tensor("xT", [E, T], HF, kind="ExternalInput").ap()
    wq_d = nc.dram_tensor("wq", [E, HPC * D], HF, kind="ExternalInput").ap()
    wk_d = nc.dram_tensor("wk", [E, HPC * D], HF, kind="ExternalInput").ap()
    wv_d = nc.dram_tensor("wv", [E, HPC * D], HF, kind="ExternalInput").ap()
    bq_d = nc.dram_tensor("bq", [128, MCH], F32, kind="ExternalInput").ap()
    bk_d = nc.dram_tensor("bk", [128, MCH], F32, kind="ExternalInput").ap()
    bv_d = nc.dram_tensor("bv", [1, HPC * D], HF, kind="ExternalInput").ap()
    wot_d = nc.dram_tensor("wot", [NEH, 128, E], HF, kind="ExternalInput").ap()
    bo_d = nc.dram_tensor("bo", [128, NEH], F32, kind="ExternalInput").ap()
    w1_d = nc.dram_tensor("w1", [NFH, 128, E], HF, kind="ExternalInput").ap()
    b1_d = nc.dram_tensor("b1", [128, NFH], F32, kind="ExternalInput").ap()
    w2_d = nc.dram_tensor("w2", [F, E], HF, kind="ExternalInput").ap()
    b2_d = nc.dram_tensor("b2", [128, NEH], F32, kind="ExternalInput").ap()
    g2c_d = nc.dram_tensor("g2c", [128, NEH], F32, kind="ExternalInput").ap()
    b2c_d = nc.dram_tensor("b2c", [128, NEH], F32, kind="ExternalInput").ap()
    xres_d = nc.dram_tensor("xres", [E, TS], F32, kind="ExternalInput").ap()
    zm_d = nc.dram_tensor("zm", [128, NC], F32, kind="ExternalInput").ap()
    cmask_d = nc.dram_tensor("cmask", [128, 4 * 512], HF, kind="ExternalInput").ap()
    onch_d = nc.dram_tensor("onch", [128, 1], HF, kind="ExternalInput").ap()
    onrr_d = nc.dram_tensor("onrr", [1, 128], F32R, kind="ExternalInput").ap()
    onrh_d = nc.dram_tensor("onrh", [1, 128], HF, kind="ExternalInput").ap()
    on4_d = nc.dram_tensor("on4", [128, HPC], HF, kind="ExternalInput").ap()
    if not causal:
        mT_d = nc.dram_tensor("mT", [S, S], F32, kind="ExternalInput").ap()
    out_d = nc.dram_tensor("out", [TS, E], F32, kind="ExternalOutput").ap()

    with tile.TileContext(nc) as tc:
        with (
            tc.tile_pool(name="res", bufs=1) as res,
            tc.tile_pool(name="wk_", bufs=2) as wrk,
            tc.tile_pool(name="st", bufs=1) as st,
            tc.tile_pool(name="ps", bufs=2, space="PSUM") as ps,
            tc.tile_pool(name="dram", bufs=1, space="DRAM") as dram,
        ):
            def rtile(name, shape, dt, tag=None):
                return res.tile(shape, dt, tag=tag or name, name=name)

            def pA(name, shape=(128, 1024)):
                return ps.tile(list(shape), F32, tag="pA", name=name, bufs=3)

            def pB(name, shape=(128, 512)):
                return ps.tile(list(shape), F32, tag="pB", name=name, bufs=2)

            # ---- constants needed by the LN1 sums pipeline go first ----
            onch = rtile("onch", [128, 1], HF)
            nc.sync.dma_start(onch[:], onch_d[:])
            onrr = rtile("onrr", [1, 128], F32R)
            nc.sync.dma_start(onrr[:], onrr_d[:])

            def late_consts():
                nonlocal onrh, on4, ident, zm, cmask, bq, bk, bv, bo, b1, b2
                nonlocal g2c, b2c, wqb, wkvb
                nc.sync.dma_start(bq[:], bq_d[:])
                nc.sync.dma_start(bk[:], bk_d[:])
                nc.sync.dma_start(bv[:], bv_d[:])
                nc.sync.dma_start(onrh[:], onrh_d[:])
                nc.sync.dma_start(on4[:], on4_d[:])
                nc.sync.dma_start(wqb[:].rearrange("p (e c) -> p e c", e=NEH),
                                  wq_d[:].rearrange("(e p) c -> p e c", p=128))
                nc.sync.dma_start(
                    wkvb[:, 0:2048].rearrange("p (e c) -> p e c", e=NEH),
                    wk_d[:].rearrange("(e p) c -> p e c", p=128))
                nc.sync.dma_start(
                    wkvb[:, 2048:4096].rearrange("p (e c) -> p e c", e=NEH),
                    wv_d[:].rearrange("(e p) c -> p e c", p=128))
                make_identity(nc, ident[:])
                nc.sync.dma_start(zm[:], zm_d[:])
                nc.sync.dma_start(cmask[:], cmask_d[:])
                nc.sync.dma_start(bo[:], bo_d[:])
                nc.sync.dma_start(b1[:], b1_d[:])
                nc.sync.dma_start(b2[:], b2_d[:])
                nc.sync.dma_start(g2c[:], g2c_d[:])
                nc.sync.dma_start(b2c[:], b2c_d[:])

            onrh = rtile("onrh", [1, 128], HF)
            on4 = rtile("on4", [128, HPC], HF)
            ident = rtile("ident", [128, 128], F32)
            zm = rtile("zm", [128, NC], F32)
            cmask = rtile("cmask", [128, 4 * 512], HF)
            bq = rtile("bq", [128, MCH], F32)
            bk = rtile("bk", [128, MCH], F32)
            bv = rtile("bv", [1, HPC * D], HF)
            bo = rtile("bo", [128, NEH], F32)
            b1 = rtile("b1", [128, NFH], F32)
            b2 = rtile("b2", [128, NEH], F32)
            g2c = rtile("g2c", [128, NEH], F32)
            b2c = rtile("b2c", [128, NEH], F32)

            # ---- resident weights/tensors ----
            wqb = rtile("wqb", [128, NEH * HPC * D], HF)        # [128, 2048]
            wkvb = rtile("wkvb", [128, 4096], HF, tag="slabW")
            qtb = rtile("qtb", [128, MCH * T], HF)              # [128, 4096]
            ktb = rtile("ktb", [128, MCH * T], HF)              # [128, 4096]
            vpb = rtile("vpb", [128, (T // 128) * HPC * 65], HF, tag="slabE")
            cfb = rtile("cfb", [128, 4096], HF)

            def vsl(tt):
                return vpb[:, 260 * tt:260 * tt + 260]

            # ---- layernorm helpers ----
            def ln_chain(pxq, width, idx):
                """DVE stats chain on accumulated [1,1024] (sum|sumsq) PSUM tile;
                returns (a_row, c_row) [1,width] f32r tiles (rstd, -mu*rstd)."""
                mean = st.tile([1, width], F32, tag="mean", name=f"mean{idx}")
                nc.vector.tensor_scalar_mul(mean[:], pxq[:, 0:width], 1.0 / E)
                msq = st.tile([1, width], F32, tag="msq", name=f"msq{idx}")
                nc.vector.tensor_scalar_mul(msq[:], pxq[:, 512:512 + width], 1.0 / E)
                scr = st.tile([1, width], F32, tag="scr", name=f"scr{idx}")
                nc.vector.tensor_tensor(scr[:], mean[:], mean[:], ALU.mult)
                var = st.tile([1, width], F32, tag="var", name=f"var{idx}")
                nc.vector.scalar_tensor_tensor(
                    out=var[:], in0=msq[:], scalar=float(EPS), in1=scr[:],
                    op0=ALU.add, op1=ALU.subtract)
                std = st.tile([1, width], F32, tag="std", name=f"std{idx}")
                nc.scalar.activation(std[:], var[:], AF.Sqrt)
                rstd = st.tile([1, width], F32, tag="rstd", name=f"rstd{idx}")
                nc.vector.reciprocal_approx_fast(rstd[:], std[:])
                a_row = st.tile([1, width], F32R, tag="a_row", name=f"a_row{idx}")
                nc.vector.tensor_copy(a_row[:], rstd[:])
                ct = st.tile([1, width], F32, tag="ct", name=f"ct{idx}")
                nc.vector.tensor_tensor(ct[:], mean[:], rstd[:], ALU.mult)
                c_row = st.tile([1, width], F32R, tag="c_row", name=f"c_row{idx}")
                nc.vector.tensor_scalar_mul(c_row[:], ct[:], -1.0)
                return a_row, c_row

            def ln_bc(a_row, c_row, width, idx):
                """Broadcast a/c rows into a bf16 SBUF tile [128, 1024]:
                [:, :512]=a, [:, 512:]=c."""
                bc = pA(f"bc{idx}")
                nc.tensor.matmul(bc[:, 0:width], onrr[:], a_row[:],
                                 start=True, stop=True)
                nc.tensor.matmul(bc[:, 512:512 + width], onrr[:], c_row[:],
                                 start=True, stop=True)
                bch = wrk.tile([128, 1024], HF, tag="bch", name=f"bch{idx}", bufs=2)
                nc.scalar.copy(bch[:, 0:width], bc[:, 0:width])
                nc.scalar.copy(bch[:, 512:512 + width], bc[:, 512:512 + width])
                return bch

            # ---- phase 1: LN1 fused with Q/K/V projections, per 512 tokens ----
            # Chunk pipeline: chunk t's zz (DVE) production is interleaved
            # per-feature-chunk with its m=0 projections and chunk t+1's LN
            # sums on PE, so neither engine waits on the other's full chunk.
            xt_all, zz_all, pxq_all, row_all = [], [], [], []

            def p1_load(tcn):
                xtc = wrk.tile([128, NEH * LW], HF, tag="xt", name=f"xt{tcn}",
                               bufs=3)
                for u in range(2):
                    nc.sync.dma_start(
                        xtc[:, 2048 * u:2048 * u + 2048]
                        .rearrange("p (e w) -> p e w", e=4),
                        xT_d[512 * u:512 * u + 512, LW * tcn:LW * tcn + LW]
                        .rearrange("(e p) w -> p e w", p=128))
                xt_all.append(xtc)

            def p1_sums_e(tcn, e):
                if e == 0:
                    pxq_all.append(pA(f"pxq{tcn}", (1, 1024)))
                pxq = pxq_all[tcn]
                src = xt_all[tcn][:, 512 * e:512 * e + 512]
                sq = wrk.tile([128, LW], HF, tag="sqh", name=f"sq{tcn}_{e}",
                              bufs=3)
                nc.scalar.activation(sq[:], src, AF.Square)
                nc.tensor.matmul(pxq[:, 512:1024], onch[:], sq[:],
                                 start=(e == 0), stop=(e == NEH - 1))
                nc.tensor.matmul(pxq[:, 0:512], onch[:], src,
                                 start=(e == 0), stop=(e == NEH - 1))

            def p1_chain(tcn):
                rows = ln_chain(pxq_all[tcn], LW, f"l1c{tcn}")
                row_all.append(rows)

            def p1_zz_e(tcn, e, bch):
                tmp = wrk.tile([128, LW], HF, tag="lnt", name=f"lnt{tcn}_{e}",
                               bufs=2)
                nc.vector.tensor_tensor(tmp[:], xt_all[tcn][:, 512 * e:512 * e + 512],
                                        bch[:, 0:LW], ALU.mult)
                zz = wrk.tile([128, LW], HF, tag="zz", name=f"zz{tcn}_{e}",
                              bufs=9)
                nc.vector.tensor_tensor(zz[:], tmp[:], bch[:, 512:512 + LW],
                                        ALU.add)
                zz_all[tcn].append(zz)

            p1_load(0)
            p1_load(1)
            late_consts()
            for e in range(NEH):
                p1_sums_e(0, e)
            p1_chain(0)
            for tcn in range(NCH):
                if tcn + 2 < NCH:
                    p1_load(tcn + 2)
                a_row, c_row = row_all[tcn]
                bch = ln_bc(a_row, c_row, LW, f"l1c{tcn}")
                zz_all.append([])
                zz_l = zz_all[tcn]
                pq0 = pB(f"pq{tcn}_0", (128, LW))
                pk0 = pB(f"pk{tcn}_0", (128, LW))
                for e in range(NEH):
                    p1_zz_e(tcn, e, bch)
                    nc.tensor.matmul(pq0[:], wqb[:, 256 * e:256 * e + 128],
                                     zz_l[e][:], start=(e == 0), stop=(e == NEH - 1))
                    nc.tensor.matmul(pk0[:], wkvb[:, 256 * e:256 * e + 128],
                                     zz_l[e][:], start=(e == 0), stop=(e == NEH - 1))
                    if tcn + 1 < NCH:
                        p1_sums_e(tcn + 1, e)
                c0 = LW * tcn
                nc.vector.tensor_scalar_add(qtb[:, c0:c0 + LW], pq0[:],
                                            bq[:, 0:1])
                nc.vector.tensor_scalar_add(ktb[:, c0:c0 + LW], pk0[:],
                                            bk[:, 0:1])
                pq1 = pB(f"pq{tcn}_1", (128, LW))
                for e in range(NEH):
                    nc.tensor.matmul(pq1[:], wqb[:, 256 * e + 128:256 * e + 256],
                                     zz_l[e][:], start=(e == 0), stop=(e == NEH - 1))
                pk1 = pB(f"pk{tcn}_1", (128, LW))
                for e in range(NEH):
                    nc.tensor.matmul(pk1[:], wkvb[:, 256 * e + 128:256 * e + 256],
                                     zz_l[e][:], start=(e == 0), stop=(e == NEH - 1))
                c1 = 2048 + LW * tcn
                nc.vector.tensor_scalar_add(qtb[:, c1:c1 + LW], pq1[:],
                                            bq[:, 1:2])
                nc.vector.tensor_scalar_add(ktb[:, c1:c1 + LW], pk1[:],
                                            bk[:, 1:2])
                for sub in range(LW // 128):
                    tt = (LW * tcn) // 128 + sub
                    pv = pB(f"pv{tt}", (128, HPC * D))
                    for e in range(NEH):
                        nc.tensor.matmul(
                            pv[:], zz_l[e][:, 128 * sub:128 * sub + 128],
                            wkvb[:, 2048 + 256 * e:2048 + 256 * e + 256],
                            start=(e == 0), stop=False)
                    nc.tensor.matmul(pv[:], onrh[:], bv[:], start=False, stop=True)
                    nc.vector.tensor_copy(
                        vsl(tt).rearrange("p (h d) -> p h d", h=HPC)[:, :, 0:D],
                        pv[:].rearrange("p (h d) -> p h d", h=HPC))
                    nc.vector.tensor_copy(
                        vsl(tt).rearrange("p (h d) -> p h d", h=HPC)[:, :, D:D + 1],
                        on4[:].rearrange("p (h o) -> p h o", o=1))
                if tcn + 1 < NCH:
                    p1_chain(tcn + 1)

            # ---- phase 3: attention (h-major; A2A #1 after heads 0-1) ----
            stageA = dram.tile([NC, 2 * D, 512], HF, tag="stageA", name="stageA")
            stageB = dram.tile([NC, 2 * D, 512], HF, tag="stageB", name="stageB")
            stage2A = dram.tile([NC, 2 * D, 512], HF, tag="stage2A", name="stage2A")
            stage2B = dram.tile([NC, 2 * D, 512], HF, tag="stage2B", name="stage2B")
            NSB = T // 512
            saA_l = []
            for h in range(HPC):
                for i in range(NSB):
                    m, hp = h // 2, h % 2
                    qsl = qtb[64 * hp:64 * hp + 64, 2048 * m + 512 * i:
                              2048 * m + 512 * i + 512]
                    n_kc = 4 * (i + 1) if causal else T // 128
                    psav = pB(f"av{h}_{i}")
                    ngr = (n_kc + 1) // 2
                    psc_l = {}

                    def emit_qk(g, h=h, i=i, m=m, hp=hp, qsl=qsl, n_kc=n_kc,
                                psc_l=psc_l):
                        kcs = [k for k in (2 * g, 2 * g + 1) if k < n_kc]
                        psc = pA(f"sc{h}_{i}_{g}")
                        for u, kc in enumerate(kcs):
                            nc.tensor.matmul(
                                psc[:, 512 * u:512 * u + 512],
                                ktb[64 * hp:64 * hp + 64,
                                    2048 * m + 128 * kc:2048 * m + 128 * kc + 128],
                                qsl, start=True, stop=True)
                        if not causal:
                            for u, kc in enumerate(kcs):
                                mb = wrk.tile([128, 512], F32, tag="mb",
                                              name=f"mb{h}_{i}_{g}_{u}", bufs=2)
                                nc.sync.dma_start(
                                    mb[:], mT_d[128 * kc:128 * kc + 128,
                                                512 * i:512 * i + 512])
                                nc.vector.tensor_tensor(
                                    psc[:, 512 * u:512 * u + 512],
                                    psc[:, 512 * u:512 * u + 512], mb[:], ALU.add)
                        psc_l[g] = (psc, kcs)

                    emit_qk(0)
                    if ngr > 1:
                        emit_qk(1)
                    for g in range(ngr):
                        if g + 2 < ngr:
                            emit_qk(g + 2)
                        psc, kcs = psc_l.pop(g)
                        eg = wrk.tile([128, 1024], HF, tag="exp",
                                      name=f"exp{h}_{i}_{g}", bufs=3)
                        w = 512 * len(kcs)
                        nc.scalar.activation(eg[:, 0:w], psc[:, 0:w], AF.Exp)
                        if causal:
                            for u, kc in enumerate(kcs):
                                j = kc - 4 * i
                                if j >= 0:
                                    nc.vector.tensor_tensor(
                                        eg[:, 512 * u:512 * u + 512],
                                        eg[:, 512 * u:512 * u + 512],
                                        cmask[:, 512 * j:512 * j + 512], ALU.mult)
                        for u, kc in enumerate(kcs):
                            nc.tensor.matmul(
                                psav[0:65, :], vsl(kc)[:, 65 * h:65 * h + 65],
                                eg[:, 512 * u:512 * u + 512],
                                start=(kc == 0), stop=(kc == n_kc - 1),
                                skip_group_check=True)
                    sden = st.tile([1, 512], F32, tag="sden", name=f"sden{h}_{i}")
                    nc.scalar.copy(sden[:], psav[64:65, :])
                    rc = st.tile([1, 512], F32, tag="rc", name=f"rc{h}_{i}")
                    nc.vector.reciprocal_approx_fast(rc[:], sden[:])
                    rch = st.tile([1, 512], HF, tag="rch", name=f"rch{h}_{i}")
                    nc.vector.tensor_copy(rch[:], rc[:])
                    rb = wrk.tile([64, 512], HF, tag="rb", name=f"rb{h}_{i}", bufs=2)
                    nc.gpsimd.partition_broadcast(rb[:], rch[:])
                    ctx = wrk.tile([64, 512], HF, tag="ctx", name=f"ctx{h}_{i}",
                                   bufs=3)
                    nc.vector.tensor_tensor(ctx[:], psav[0:64, :], rb[:], ALU.mult)
                    stg = stageA if h < 2 else stageB
                    hh = h % 2
                    for s_ in (i, i + 4):
                        cz = wrk.tile([64, 512], HF, tag="cz",
                                      name=f"cz{h}_{i}_{s_}", bufs=3)
                        nc.vector.tensor_scalar_mul(cz[:], ctx[:],
                                                    zm[0:64, s_:s_ + 1])
                        nc.sync.dma_start(stg[s_][64 * hh:64 * hh + 64, :], cz[:])
                    if h == 3 and i == 1:
                        # A2A#1 results: DMA-only on the gpsimd queue, placed
                        # here so head 3's remaining partition_broadcasts are
                        # not stuck behind the collective-completion wait
                        for s in range(4):
                            sa = wrk.tile([128, 512], HF, tag="sa",
                                          name=f"saA{s}", bufs=8)
                            nc.gpsimd.dma_start(sa[:], stage2A[s])
                            sb_ = wrk.tile([128, 512], HF, tag="sa",
                                           name=f"sbA{s}", bufs=8)
                            nc.gpsimd.dma_start(sb_[:], stage2A[4 + s])
                            saA_l.append((sa, sb_))
                if h == 1:
                    nc.gpsimd.collective_compute(
                        "AllToAll", ALU.bypass, replica_groups=[list(range(NC))],
                        ins=[stageA.opt()], outs=[stage2A.opt()])
            nc.gpsimd.collective_compute(
                "AllToAll", ALU.bypass, replica_groups=[list(range(NC))],
                ins=[stageB.opt()], outs=[stage2B.opt()])
            saB_l = []
            for s in range(4):
                sa = wrk.tile([128, 512], HF, tag="sa", name=f"saB{s}", bufs=8)
                nc.gpsimd.dma_start(sa[:], stage2B[s])
                sb_ = wrk.tile([128, 512], HF, tag="sa", name=f"sbB{s}", bufs=8)
                nc.gpsimd.dma_start(sb_[:], stage2B[4 + s])
                saB_l.append((sa, sb_))

            # ---- phase 4: out-proj (two passes) + residual + LN2 ----
            # cfb chunk s (s<4): features of heads 0-1 of source s;
            # chunk 4+s: features of heads 2-3 of source s.
            yb = rtile("yb", [128, 4096], HF)
            for s in range(4):
                sa, sb_ = saA_l[s]
                nc.vector.tensor_tensor(cfb[:, 512 * s:512 * s + 512],
                                        sa[:], sb_[:], ALU.add)
            for eo in range(NEH):
                wof = wrk.tile([128, E], HF, tag="ws", name=f"wofA{eo}", bufs=3)
                nc.sync.dma_start(wof[:], wot_d[eo])
                po = pB(f"poA{eo}")
                for s in range(4):
                    nc.tensor.matmul(po[:], wof[:, 256 * s:256 * s + 128],
                                     cfb[:, 512 * s:512 * s + 512],
                                     start=(s == 0), stop=(s == 3))
                xrs = wrk.tile([128, 512], F32, tag="xrs", name=f"xrs{eo}", bufs=2)
                nc.sync.dma_start(xrs[:], xres_d[128 * eo:128 * eo + 128, :])
                nc.vector.scalar_tensor_tensor(
                    out=yb[:, 512 * eo:512 * eo + 512], in0=po[:],
                    scalar=bo[:, eo:eo + 1], in1=xrs[:],
                    op0=ALU.add, op1=ALU.add)
            for s in range(4):
                sa, sb_ = saB_l[s]
                nc.vector.tensor_tensor(cfb[:, 2048 + 512 * s:2048 + 512 * s + 512],
                                        sa[:], sb_[:], ALU.add)
            pxq2 = pA("pxq2", (1, 1024))
            sq2_l = []
            for eo in range(NEH):
                wof = wrk.tile([128, E], HF, tag="ws", name=f"wofB{eo}", bufs=3)
                nc.sync.dma_start(wof[:], wot_d[eo])
                po = pB(f"poB{eo}")
                for s in range(4):
                    nc.tensor.matmul(po[:], wof[:, 256 * s + 128:256 * s + 256],
                                     cfb[:, 2048 + 512 * s:2048 + 512 * s + 512],
                                     start=(s == 0), stop=(s == 3))
                ybe = yb[:, 512 * eo:512 * eo + 512]
                nc.vector.tensor_tensor(ybe, po[:], ybe, ALU.add)
                # LN2 sums interleaved with out-proj pass B
                sq = wrk.tile([128, 512], HF, tag="sqh", name=f"sq2_{eo}",
                              bufs=3)
                nc.scalar.activation(sq[:], ybe, AF.Square)
                nc.tensor.matmul(pxq2[:, 512:1024], onch[:], sq[:],
                                 start=(eo == 0), stop=(eo == NEH - 1))
                nc.tensor.matmul(pxq2[:, 0:512], onch[:], ybe,
                                 start=(eo == 0), stop=(eo == NEH - 1))
                sq2_l.append(sq)
            # LN2 stats chain
            a_row2, c_row2 = ln_chain(pxq2, 512, "l2")
            bch2 = ln_bc(a_row2, c_row2, 512, "l2")
            x2r = rtile("x2rb", [128, 4096], HF, tag="slabW")
            for e in range(NEH):
                w0 = 512 * e
                tmp = wrk.tile([128, 512], HF, tag="lnt", name=f"lnt2_{e}", bufs=2)
                nc.vector.tensor_tensor(tmp[:], yb[:, w0:w0 + 512],
                                        bch2[:, 0:512], ALU.mult)
                tm2 = wrk.tile([128, 512], HF, tag="lnu", name=f"lnu2_{e}", bufs=2)
                nc.vector.tensor_tensor(tm2[:], tmp[:], bch2[:, 512:1024], ALU.add)
                nc.vector.tensor_scalar(
                    out=x2r[:, w0:w0 + 512], in0=tm2[:],
                    scalar1=g2c[:, e:e + 1], scalar2=b2c[:, e:e + 1],
                    op0=ALU.mult, op1=ALU.add)

            # ---- phase 5: FFN (fc1/fc2 interleaved by f-group) ----
            outT = rtile("outTb", [128, 4096], F32, tag="slabF")
            for grp in range(NFH // FG):
                fs = list(range(FG * grp, FG * (grp + 1)))
                h1_l = []
                for f in fs:
                    w1f = wrk.tile([128, E], HF, tag="ws", name=f"w1f{f}", bufs=3)
                    nc.sync.dma_start(w1f[:], w1_d[f])
                    pf = pA(f"pf{f}", (128, 512))
                    for e in range(NEH):
                        nc.tensor.matmul(pf[:], w1f[:, 128 * e:128 * e + 128],
                                         x2r[:, 512 * e:512 * e + 512],
                                         start=(e == 0), stop=(e == NEH - 1))
                    h1f = wrk.tile([128, 512], HF, tag="h1g", name=f"h1_{f}",
                                   bufs=FG + 2)
                    nc.scalar.activation(h1f[:], pf[:], AF.Gelu, bias=b1[:, f:f + 1])
                    h1_l.append(h1f)
                w2f_l = []
                for f in fs:
                    w2f = wrk.tile([128, E], HF, tag="w2s", name=f"w2f{f}",
                                   bufs=FG)
                    nc.sync.dma_start(w2f[:], w2_d[128 * f:128 * f + 128, :])
                    w2f_l.append(w2f)
                for eo in range(NEH):
                    p2 = pB(f"p2_{grp}_{eo}")
                    for u in range(FG):
                        nc.tensor.matmul(p2[:], w2f_l[u][:, 128 * eo:128 * eo + 128],
                                         h1_l[u][:], start=(u == 0),
                                         stop=(u == FG - 1))
                    o_ = outT[:, 512 * eo:512 * eo + 512]
                    if grp == 0:
                        nc.vector.tensor_copy(o_, p2[:])
                    else:
                        nc.vector.tensor_tensor(o_, p2[:], o_, ALU.add)
                    if grp == NFH // FG - 1:
                        nc.vector.scalar_tensor_tensor(
                            out=o_, in0=o_, scalar=b2[:, eo:eo + 1],
                            in1=x2r[:, 512 * eo:512 * eo + 512],
                            op0=ALU.add, op1=ALU.add)

            # ---- phase 6: transpose to [tokens, E] and store ----
            for ts_ in range(TS // 128):
                osb = wrk.tile([128, E], F32, tag="osb", name=f"osb{ts_}", bufs=2)
                for eo in range(NEH):
                    pt = pB(f"pt{ts_}_{eo}", (128, 128))
                    nc.tensor.transpose(
                        pt[:],
                        outT[:, 512 * eo + 128 * ts_:512 * eo + 128 * ts_ + 128],
                        ident[:])
                    nc.vector.tensor_copy(osb[:, 128 * eo:128 * eo + 128], pt[:])
                nc.sync.dma_start(out_d[128 * ts_:128 * ts_ + 128, :], osb[:])

    nc.compile()
    return nc


_CACHE = {}


def _get_nc(causal):
    if causal not in _CACHE:
        _CACHE[causal] = build(causal)
    return _CACHE[causal]


def _build_in_maps(x, mask, Wq, bq, Wk, bk, Wv, bv, Wo, bo, W1, b1, W2, b2,
                   ln1_g, ln1_b, ln2_g, ln2_b):
    x = np.asarray(x, np.float32)
    mask2d = np.asarray(mask).reshape(S, S)
    causal = bool(np.array_equal(mask2d, np.tril(np.ones((S, S), mask2d.dtype))))

    def colmaj(v, nch):
        return np.ascontiguousarray(np.asarray(v, np.float32).reshape(nch, 128).T)

    def bf(a):
        return np.ascontiguousarray(np.asarray(a, np.float32).astype(BF_NP))

    ln1_g = np.asarray(ln1_g, np.float32)
    ln1_b = np.asarray(ln1_b, np.float32)
    Wq = np.asarray(Wq, np.float32)
    Wk = np.asarray(Wk, np.float32)
    Wv = np.asarray(Wv, np.float32)
    Wo = np.asarray(Wo, np.float32)
    W1 = np.asarray(W1, np.float32)
    Wqf = Wq * ln1_g[:, None] / np.sqrt(D)
    bqf = (ln1_b @ Wq + np.asarray(bq, np.float32)) / np.sqrt(D)
    Wkf = Wk * ln1_g[:, None]
    bkf = ln1_b @ Wk + np.asarray(bk, np.float32)
    Wvf = Wv * ln1_g[:, None]
    bvf = ln1_b @ Wv + np.asarray(bv, np.float32)
    w1t = np.ascontiguousarray(
        W1.reshape(NEH, 128, NFH, 128).transpose(2, 1, 0, 3).reshape(NFH, 128, E))
    # wot chunk s: rows = heads (2s%?) -- feature-major [contract 128, E out]
    wot = np.ascontiguousarray(
        Wo.reshape(NEH, 128, NEH, 128).transpose(2, 1, 0, 3).reshape(NEH, 128, E))

    cm = np.zeros((128, 4 * 512), np.float32)
    for j in range(4):
        p, q = np.meshgrid(np.arange(128), np.arange(512), indexing='ij')
        cm[:, 512 * j:512 * (j + 1)] = (q >= p + 128 * j).astype(np.float32)

    xT = [np.ascontiguousarray(x[b_].T) for b_ in range(B)]
    xTh = [bf(a) for a in xT]
    shared = {
        "wot": bf(wot), "bo": colmaj(bo, NEH),
        "w1": bf(w1t), "b1": colmaj(b1, NFH),
        "w2": bf(np.asarray(W2, np.float32)),
        "b2": colmaj(b2, NEH),
        "g2c": colmaj(ln2_g, NEH),
        "b2c": colmaj(ln2_b, NEH),
        "cmask": bf(cm),
        "onch": np.ones((128, 1), BF_NP),
        "onrr": np.ones((1, 128), np.float32),
        "onrh": np.ones((1, 128), BF_NP),
        "on4": np.ones((128, HPC), BF_NP),
    }
    if not causal:
        shared["mT"] = np.ascontiguousarray(
            np.where(mask2d == 0, np.float32(-1e9), np.float32(0.0)).T)
    zms = []
    for b_ in range(B):
        z_ = np.zeros((128, NC), np.float32)
        z_[:, 4 * b_:4 * b_ + 4] = 1.0
        zms.append(z_)

    in_maps = []
    for c in range(NC):
        b_, j = c // 4, c % 4
        cs = slice(HPC * D * j, HPC * D * (j + 1))
        m = {
            "xT": xTh[b_], "zm": zms[b_],
            "xres": np.ascontiguousarray(xT[b_][:, TS * j:TS * (j + 1)]),
            "wq": bf(Wqf[:, cs]),
            "wk": bf(Wkf[:, cs]),
            "wv": bf(Wvf[:, cs]),
            "bq": colmaj(bqf[cs], MCH),
            "bk": colmaj(bkf[cs], MCH),
            "bv": bf(bvf[None, cs]),
        }
        m.update(shared)
        in_maps.append(m)
    return in_maps, causal


def kernel(**inputs):
    in_maps, causal = _build_in_maps(**inputs)
    nc_obj = _get_nc(causal)
    res = run_bass_kernel_spmd(nc_obj, in_maps, list(range(NC)))
    out = np.empty((B, S, E), np.float32)
    for c in range(NC):
        b_, j = c // 4, c % 4
        out[b_, TS * j:TS * (j + 1), :] = res.results[c]["out"]
    return out
